# revision 12
# baseline (speedup 1.0000x reference)
"""CosFace loss (B=1024, D=512, C=100000) on 8 Trainium2 NeuronCores.

Strategy (tensor-parallel classification head, per sharding hint):
  - Classes sharded 12500/core (padded to 12544 = 98*128 with zero rows).
  - fp8 (e4m3) matmul in DoubleRow perf mode: both operands carry two
    128-row contraction slices per instruction (effective K=256), which
    runs the PE at 2x the bf16/f32r rate and cuts weight DMA 4x vs f32.
    Weights are pre-scaled by 64 and x by 8 on the host so fp8 normals
    cover the value range; the scales divide out in the exp scale.
  - Host prep: weight rows L2-normalized, scaled, cast to fp8, shard
    transposed to [D, C_loc]; x transposed/cast; per-row label-cosine
    terms (dneg, t2) and exp scales (scl) computed exactly on host.
  - Device per core: stream wT chunks, DoubleRow matmuls accumulate
    cosine tiles [128b, CHUNK] in PSUM; fused ScalarE pass computes
    exp(scl_b*psum - 64) with accum_out giving per-row partial
    sum-of-exp (the fixed shift 64 = S bounds |logits|, so no running
    max is needed: exp args are in [-128, ~0], safely inside fp32).
  - One 4KB AllReduce of the [1024] partial sums.
  - Margin fixup: the label logit must be S*(cos-M), not S*cos.  The
    host computes dneg = e^(S*cos_l-64) - e^(S*cos_l-S*M-64) and
    t2 = S*cos_l - S*M - 64 + 66*ln2; the device does
    adj = AR_sum - dneg, loss_row = ln(adj*2^66) - t2, loss =
    mean(loss_row).  Core 0's output is used.
"""

import numpy as np

import concourse.bass as bass
import concourse.mybir as mybir
import concourse.tile as tile
from concourse import bacc
from concourse.bass_utils import run_bass_kernel_spmd

B, D, C = 1024, 512, 100000
S, MARGIN = 64.0, 0.35
SHIFT = 64.0
NCORES = 8
CSHARD = C // NCORES          # 12500 real classes per core
CLOC = 12544                  # padded (98 * 128)
KT2 = D // 256                # 2 double-row contraction tiles
MT = B // 128                 # 8 batch tiles
SCALE_X = 8.0
SCALE_W = 64.0
LN2_66 = 66.0 * float(np.log(2.0))
import os as _os
CHUNK = int(_os.environ.get('KCHUNK', '2048'))   # classes per PSUM group
NCHUNK = (CLOC + CHUNK - 1) // CHUNK
NMM = int(_os.environ.get('NMM', '512'))         # out columns per matmul
WBUFS = int(_os.environ.get('WBUFS', '3'))
PBUFS = int(_os.environ.get('PBUFS', '2'))

F32 = mybir.dt.float32
F8 = mybir.dt.float8e4
AF = mybir.ActivationFunctionType
AX = mybir.AxisListType
ALU = mybir.AluOpType
DR = mybir.MatmulPerfMode.DoubleRow

_NC = None
LAST_RESULTS = None
ABLATE = _os.environ.get('ABLATE', 'full')  # full | noexp | nomm | nodma
TAIL = _os.environ.get('TAIL', 'host')      # host | device


def _body(nc, tc, xt, wt, scl, dneg, t2, loss, collective=True):
    from contextlib import ExitStack
    with ExitStack() as ctx:
        singles = ctx.enter_context(tc.tile_pool(name="singles", bufs=1))
        wpool = ctx.enter_context(tc.tile_pool(name="wpool", bufs=WBUFS))
        psump = ctx.enter_context(tc.tile_pool(name="psump", bufs=PBUFS, space="PSUM"))
        dram = ctx.enter_context(tc.tile_pool(name="dram", bufs=1, space="DRAM"))
        wt_v = wt.ap().rearrange("(k i p) c -> p k i c", p=128, i=2)
        # ---- first weight chunk before anything else: it gates the PE ----
        wt_cs = []
        for c in range(NCHUNK):
            wt_chunk_tile = wpool.tile([128, KT2, 2, CHUNK], F8, tag="wt")
            wt_cs.append(wt_chunk_tile)
        if ABLATE != 'nodma':
            nc.sync.dma_start(out=wt_cs[0][:, :, :, :CHUNK],
                              in_=wt_v[:, :, :, 0:CHUNK])
        # ---- resident inputs ----
        xt_sb = singles.tile([128, KT2, 2, B], F8)
        nc.sync.dma_start(out=xt_sb[:, :, :, :],
                          in_=xt.ap().rearrange("(k i p) b -> p k i b", p=128, i=2))
        # small vectors ride the DVE queue; they are not needed until the exp
        scl_sb = singles.tile([128, MT], F32)
        nc.gpsimd.dma_start(out=scl_sb[:, :], in_=scl.ap()[:, :])
        if TAIL != 'host':
            dneg_sb = singles.tile([128, MT], F32)
            nc.gpsimd.dma_start(out=dneg_sb[:, :], in_=dneg.ap()[:, :])
            t2_sb = singles.tile([128, MT], F32)
            nc.gpsimd.dma_start(out=t2_sb[:, :], in_=t2.ap()[:, :])

        # const bias column (activation bias must be a [P,1] AP)
        cb_m64 = singles.tile([128, 1], F32)
        nc.vector.memset(cb_m64[:, :], -SHIFT)

        # ---- main loop: cosine matmuls + fused exp/accumulate ----
        sums = singles.tile([128, MT, NCHUNK], F32)
        nc.vector.memset(sums[:, :, :], 0.0)
        for c in range(NCHUNK):
            c0 = c * CHUNK
            ncls = min(CHUNK, CLOC - c0)
            wt_c = wt_cs[c]
            if ABLATE != 'nodma' and c > 0:
                nc.sync.dma_start(out=wt_c[:, :, :, :ncls],
                                  in_=wt_v[:, :, :, c0:c0 + ncls])
            for m in range(MT):
                g = psump.tile([128, CHUNK], F32, tag="g")
                if ABLATE != 'nomm':
                    for k in range(KT2):
                        lhsT = xt_sb[:, k, :, m * 128:(m + 1) * 128]
                        for n in range(0, ncls, NMM):
                            nsz = min(NMM, ncls - n)
                            nc.tensor.matmul(g[:, n:n + nsz], lhsT,
                                             wt_c[:, k, :, n:n + nsz],
                                             start=(k == 0), stop=(k == KT2 - 1),
                                             perf_mode=DR)
                if ABLATE != 'noexp':
                    # in-place on PSUM: we only need accum_out; ScalarE is
                    # closest to PSUM and this avoids an SBUF scratch write
                    nc.scalar.activation(g[:, :ncls], g[:, :ncls], AF.Exp,
                                         bias=cb_m64[:, :], scale=scl_sb[:, m:m + 1],
                                         accum_out=sums[:, m, c:c + 1])

        # ---- reduce partials + AllReduce ----
        se_part = singles.tile([128, MT], F32)
        nc.vector.tensor_reduce(se_part[:, :], sums[:, :, :], axis=AX.X, op=ALU.add)
        if TAIL == 'host':
            # ship per-core partial sums; the host sums the 8 cores and
            # finishes logz/mean (microseconds of CPU in the unshard step)
            nc.sync.dma_start(out=loss.ap()[:, :], in_=se_part[:, :])
            return
        full_se = singles.tile([128, MT], F32)
        if collective:
            ar_in = dram.tile([128, MT], F32)
            ar_out = dram.tile([128, MT], F32, addr_space="Shared")
            nc.sync.dma_start(out=ar_in[:, :], in_=se_part[:, :])
            nc.gpsimd.collective_compute(
                "AllReduce", ALU.add,
                replica_groups=[list(range(NCORES))],
                ins=[ar_in.opt()], outs=[ar_out.opt()])
            nc.sync.dma_start(out=full_se[:, :], in_=ar_out[:, :])
        else:
            nc.vector.tensor_scalar_mul(full_se[:, :], se_part[:, :], float(NCORES))

        # ---- logz and loss ----
        adj = singles.tile([128, MT], F32)
        nc.vector.tensor_sub(adj[:, :], full_se[:, :], dneg_sb[:, :])
        ln_adj = singles.tile([128, MT], F32)
        # ACT Ln is inaccurate for tiny args (~1e-21); prescale into [0.01, 10]
        # via the free affine input (ln(adj*2^66) = ln(adj) + 66*ln2, the
        # constant is folded into t2).
        nc.scalar.activation(ln_adj[:, :], adj[:, :], AF.Ln, scale=float(2.0 ** 66))
        lossv = singles.tile([128, MT], F32)
        nc.vector.tensor_sub(lossv[:, :], ln_adj[:, :], t2_sb[:, :])
        rowsum = singles.tile([128, 1], F32)
        junk2 = singles.tile([128, MT], F32)
        nc.scalar.activation(junk2[:, :], lossv[:, :], AF.Identity,
                             accum_out=rowsum[:, :])
        # partition-axis reduce on PE: [1,1] = ones.T @ rowsum
        ones_col = singles.tile([128, 1], F32)
        nc.vector.memset(ones_col[:, :], 1.0)
        fin_ps = psump.tile([1, 1], F32, tag="g")
        nc.tensor.matmul(fin_ps[:, :], ones_col[:, :], rowsum[:, :],
                         start=True, stop=True)
        fin = singles.tile([1, 1], F32)
        nc.scalar.activation(fin[:, :], fin_ps[:, :], AF.Identity,
                             scale=1.0 / B)
        nc.sync.dma_start(out=loss.ap()[:, :], in_=fin[:, :])


def _build(repeat=1, collective=True):
    nc = bacc.Bacc("TRN2", target_bir_lowering=False, debug=False,
                   num_devices=NCORES)
    xt = nc.dram_tensor("xt", [D, B], F8, kind="ExternalInput")
    wt = nc.dram_tensor("wt", [D, CLOC], F8, kind="ExternalInput")
    scl = nc.dram_tensor("scl", [128, MT], F32, kind="ExternalInput")
    if TAIL == 'host':
        dneg = t2 = None
        loss = nc.dram_tensor("loss", [128, MT], F32, kind="ExternalOutput")
    else:
        dneg = nc.dram_tensor("dneg", [128, MT], F32, kind="ExternalInput")
        t2 = nc.dram_tensor("t2", [128, MT], F32, kind="ExternalInput")
        loss = nc.dram_tensor("loss", [1, 1], F32, kind="ExternalOutput")
    with tile.TileContext(nc) as tc:
        for _ in range(repeat):
            _body(nc, tc, xt, wt, scl, dneg, t2, loss, collective=collective)
    nc.compile()
    return nc


def _get_nc():
    global _NC
    if _NC is None:
        _NC = _build()
    return _NC


def _to_pcol(v):
    """[B] -> [128, MT] with b = m*128 + p at [p, m]."""
    return np.ascontiguousarray(np.asarray(v, dtype=np.float32).reshape(MT, 128).T)


def _prep(inputs):
    f8 = mybir.dt.np(F8)
    x = np.asarray(inputs["input"], dtype=np.float64)
    label = np.asarray(inputs["label"]).astype(np.int64)
    w = np.asarray(inputs["weight"], dtype=np.float64)
    wn = w / np.maximum(np.sqrt((w * w).sum(axis=1, keepdims=True)), 1e-12)
    xnorm = np.maximum(np.sqrt((x * x).sum(axis=1)), 1e-12)
    cos_l = (x * wn[label]).sum(axis=1) / xnorm
    scl = S / (SCALE_X * SCALE_W * xnorm)
    dneg = np.exp(S * cos_l - SHIFT) - np.exp(S * cos_l - S * MARGIN - SHIFT)
    t2 = S * cos_l - S * MARGIN - SHIFT + LN2_66
    xt8 = np.clip(x.T * SCALE_X, -224.0, 224.0).astype(f8)
    xt8 = np.ascontiguousarray(xt8)
    scl_c = _to_pcol(scl)
    dneg_c = _to_pcol(dneg)
    t2_c = _to_pcol(t2)
    in_maps = []
    for k in range(NCORES):
        shard = np.zeros((D, CLOC), dtype=f8)
        blk = np.clip(wn[k * CSHARD:(k + 1) * CSHARD].T * SCALE_W, -224.0, 224.0)
        shard[:, :CSHARD] = blk.astype(f8)
        m = {"xt": xt8, "wt": shard, "scl": scl_c}
        if TAIL != 'host':
            m["dneg"] = dneg_c
            m["t2"] = t2_c
        in_maps.append(m)
    return in_maps


def _host_tail(inputs, partials):
    """partials: list of [128, MT] per-core sum-of-exp partials."""
    x = np.asarray(inputs["input"], dtype=np.float64)
    label = np.asarray(inputs["label"]).astype(np.int64)
    w = np.asarray(inputs["weight"], dtype=np.float64)
    wl = w[label]
    wln = wl / np.maximum(np.sqrt((wl * wl).sum(axis=1, keepdims=True)), 1e-12)
    xnorm = np.maximum(np.sqrt((x * x).sum(axis=1)), 1e-12)
    cos_l = (x * wln).sum(axis=1) / xnorm
    se = np.zeros(B, dtype=np.float64)
    for p in partials:
        se += np.asarray(p, dtype=np.float64).T.reshape(B)
    dneg = np.exp(S * cos_l - SHIFT) - np.exp(S * cos_l - S * MARGIN - SHIFT)
    lossrow = np.log(se - dneg) + SHIFT - (S * cos_l - S * MARGIN)
    return np.float32(lossrow.mean())


def kernel(**inputs):
    global LAST_RESULTS
    # this axon client build has no NTFF hook; a stray BASS_TRACE=1 in the
    # environment would crash run_bass_kernel_spmd on an optional import
    _os.environ["BASS_NEVER_TRACE"] = "1"
    nc = _get_nc()
    in_maps = _prep(inputs)
    res = run_bass_kernel_spmd(nc, in_maps, core_ids=list(range(NCORES)))
    LAST_RESULTS = res
    if TAIL == 'host':
        partials = [res.results[k]["loss"] for k in range(NCORES)]
        return np.asarray(_host_tail(inputs, partials), dtype=np.float32)
    return np.asarray(res.results[0]["loss"][0, 0], dtype=np.float32)
